# revision 3
# baseline (speedup 1.0000x reference)
"""Trainium2 Bass kernel for nn_BaseGraph_67697274519895 (gnn_message_passing).

Reference computation (B=8, N=256, D=128, E=65280):
    edge_feat = concat([x[:, recv, :], x[:, send, :]], -1)        # [B, E, 2D]
    out = zeros([B, N, 2D]).at[:, recv, :].add(edge_feat) / N

With R/S the one-hot [E, N] incidence matrices of recv/send, the scatter-add
is out = R^T @ concat(R @ x, S @ x) / N, which collapses algebraically:
    out[:, :, :D]  = (R^T R) @ x / N = diag(bincount(recv)) @ x / N
    out[:, :, D:]  = (R^T S) @ x / N = A @ x / N,  A[i, j] = #edges (r=i, s=j)
Valid for arbitrary index arrays. A and the counts are built host-side from
the indices (O(E) bincount); the device runs, per batch element, a
[N, N] @ [N, D] matmul plus a per-node row scale.

Sharding: data-parallel over batch — core b handles x[b]; A and counts are
replicated to all 8 cores. No collectives.
"""

import numpy as np

B, N, D = 8, 256, 128
N_CORES = 8
P = 128  # SBUF partitions

_PROGRAM = None  # cached (nc) Bass program


def _build_program():
    import concourse.mybir as mybir
    from concourse import bacc
    from concourse.tile import TileContext

    f32 = mybir.dt.float32
    nc = bacc.Bacc(trn_type="TRN2")

    x = nc.dram_tensor("x", [N, D], f32, kind="ExternalInput")
    atc = nc.dram_tensor("atc", [N, N], f32, kind="ExternalInput")  # A^T / N
    cnt = nc.dram_tensor("cnt", [N, 1], f32, kind="ExternalInput")  # counts / N
    out = nc.dram_tensor("out", [N, 2 * D], f32, kind="ExternalOutput")

    KB = N // P  # contraction blocks
    IB = N // P  # output row blocks

    with TileContext(nc) as tc:
        with (
            tc.tile_pool(name="sbuf", bufs=1) as pool,
            tc.tile_pool(name="psum", bufs=1, space="PSUM") as psum_pool,
        ):
            xt = []
            for k in range(KB):
                t = pool.tile([P, D], f32, tag=f"x{k}", name=f"x{k}")
                nc.sync.dma_start(out=t[:], in_=x[k * P : (k + 1) * P, :])
                xt.append(t)

            att = []
            for k in range(KB):
                row = []
                for i in range(IB):
                    t = pool.tile([P, P], f32, tag=f"at{k}{i}", name=f"at{k}{i}")
                    nc.sync.dma_start(
                        out=t[:], in_=atc[k * P : (k + 1) * P, i * P : (i + 1) * P]
                    )
                    row.append(t)
                att.append(row)

            cts = []
            for i in range(IB):
                t = pool.tile([P, 1], f32, tag=f"cnt{i}", name=f"cnt{i}")
                nc.sync.dma_start(out=t[:], in_=cnt[i * P : (i + 1) * P, :])
                cts.append(t)

            for i in range(IB):
                ps = psum_pool.tile([P, D], f32, tag=f"ps{i}", name=f"ps{i}")
                for k in range(KB):
                    # (A/N @ x)[i-block] = sum_k (A^T/N)[k-block, i-block].T @ x[k-block]
                    nc.tensor.matmul(
                        ps[:],
                        att[k][i][:],
                        xt[k][:],
                        start=(k == 0),
                        stop=(k == KB - 1),
                    )
                ot = pool.tile([P, 2 * D], f32, tag=f"o{i}", name=f"o{i}")
                # first D features: x * (recv_count / N), per-partition scalar
                nc.vector.tensor_scalar_mul(ot[:, 0:D], xt[i][:], cts[i][:])
                # last D features: A @ x / N from PSUM
                nc.vector.tensor_copy(ot[:, D : 2 * D], ps[:])
                nc.sync.dma_start(out=out[i * P : (i + 1) * P, :], in_=ot[:])

    nc.compile()
    return nc


def kernel(x, receivers, senders):
    global _PROGRAM
    from concourse.bass_utils import run_bass_kernel_spmd

    x = np.ascontiguousarray(np.asarray(x), dtype=np.float32)
    recv = np.asarray(receivers).astype(np.int64).ravel()
    send = np.asarray(senders).astype(np.int64).ravel()
    assert x.shape == (B, N, D), x.shape

    # A^T[s, r] = #edges with (receiver=r, sender=s); scaled by 1/N (exact, N=2^8)
    atc = (
        np.bincount(send * N + recv, minlength=N * N)
        .reshape(N, N)
        .astype(np.float32)
        / N
    )
    cnt = (np.bincount(recv, minlength=N).astype(np.float32) / N).reshape(N, 1)

    if _PROGRAM is None:
        _PROGRAM = _build_program()
    nc = _PROGRAM

    in_maps = [
        {"x": np.ascontiguousarray(x[b]), "atc": atc, "cnt": cnt} for b in range(B)
    ]
    res = run_bass_kernel_spmd(nc, in_maps, core_ids=list(range(N_CORES)))
    return np.stack([res.results[b]["out"] for b in range(B)], axis=0)


# revision 5
# speedup vs baseline: 1.2647x; 1.2647x over previous
"""Trainium2 Bass kernel for nn_BaseGraph_67697274519895 (gnn_message_passing).

Reference computation (B=8, N=256, D=128, E=65280):
    edge_feat = concat([x[:, recv, :], x[:, send, :]], -1)        # [B, E, 2D]
    out = zeros([B, N, 2D]).at[:, recv, :].add(edge_feat) / N

With R/S the one-hot [E, N] incidence matrices of recv/send, the scatter-add
is out = R^T @ concat(R @ x, S @ x) / N, which collapses algebraically:
    out[:, :, :D]  = (R^T R) @ x / N = diag(bincount(recv)) @ x / N
    out[:, :, D:]  = (R^T S) @ x / N = A @ x / N,  A[i, j] = #edges (r=i, s=j)
Valid for arbitrary index arrays. A and the counts are built host-side from
the indices (O(E) bincount); the device runs, per batch element, a
[N, N] @ [N, D] matmul plus a per-node row scale.

Sharding: data-parallel over batch — core b handles x[b]; A and counts are
replicated to all 8 cores. No collectives.

Device-side layout (tuned against the TRN2 instruction cost model — DMA
fixed costs dominate at this size, so everything is packed to minimize DMA
instruction count and maximize per-descriptor contiguity):
  - ONE packed input tensor inp[2, 128, 385]: row r of block k holds
    [x[b, 128k+r, :] | (A^T/N)[128k+r, :] | cnt[128k+r]/N].  2 input DMAs.
  - matmul transposed: psum[d, n] = sum_k x_k^T @ (A^T)_k  (free dim 256,
    single PSUM accumulation group), DMA'd straight from PSUM to DRAM.
  - x*cnt half via one DVE tensor-scalar multiply per k block into a single
    [128, 2, 128] tile, one DMA out.  Host un-transposes/interleaves.
"""

import numpy as np

B, N, D = 8, 256, 128
N_CORES = 8
P = 128
W = D + N + 1  # packed input row: x | A^T/N | cnt/N

MATMUL_F32R = False  # flip to use float32r matmuls (4x faster, lower precision)

_PROGRAM = None


def _build_program():
    import concourse.mybir as mybir
    from concourse import bacc
    from concourse.tile import TileContext

    f32 = mybir.dt.float32
    nc = bacc.Bacc(trn_type="TRN2")

    inp = nc.dram_tensor("inp", [2, P, W], f32, kind="ExternalInput")
    o1 = nc.dram_tensor("o1", [P, 2, D], f32, kind="ExternalOutput")
    o2t = nc.dram_tensor("o2t", [D, N], f32, kind="ExternalOutput")

    with TileContext(nc) as tc:
        with (
            tc.tile_pool(name="sbuf", bufs=1) as pool,
            tc.tile_pool(name="psum", bufs=1, space="PSUM") as psum_pool,
        ):
            tk = []
            for k in range(2):
                t = pool.tile([P, W], f32, tag=f"in{k}", name=f"in{k}")
                nc.sync.dma_start(out=t[:], in_=inp[k])
                tk.append(t)

            # out2^T[d, n] = sum over sender blocks k of x_k^T @ (A^T)_k
            ps = psum_pool.tile([P, N], f32, name="ps")
            for k in range(2):
                lhsT = tk[k][:, 0:D]  # [128 senders, 128 feat] (stationary)
                rhs = tk[k][:, D : D + N]  # [128 senders, 256 receivers]
                if MATMUL_F32R:
                    lhsT = lhsT.bitcast(mybir.dt.float32r)
                    rhs = rhs.bitcast(mybir.dt.float32r)
                nc.tensor.matmul(ps[:], lhsT, rhs, start=(k == 0), stop=(k == 1))

            # out1[p, k, :] = x[128k+p, :] * cnt[128k+p]/N
            ot1 = pool.tile([P, 2, D], f32, name="ot1")
            for k in range(2):
                nc.vector.tensor_scalar_mul(
                    ot1[:, k, :], tk[k][:, 0:D], tk[k][:, W - 1 : W]
                )

            nc.sync.dma_start(out=o1[:], in_=ot1[:])
            ot2 = pool.tile([P, N], f32, name="ot2")
            nc.vector.tensor_copy(ot2[:], ps[:])
            nc.sync.dma_start(out=o2t[:], in_=ot2[:])

    nc.compile()
    return nc


def kernel(x, receivers, senders):
    global _PROGRAM
    from concourse.bass_utils import run_bass_kernel_spmd

    x = np.ascontiguousarray(np.asarray(x), dtype=np.float32)
    recv = np.asarray(receivers).astype(np.int64).ravel()
    send = np.asarray(senders).astype(np.int64).ravel()
    assert x.shape == (B, N, D), x.shape

    # A^T[s, r] = #edges with (receiver=r, sender=s); scaled by 1/N (exact, N=2^8)
    atc = (
        np.bincount(send * N + recv, minlength=N * N)
        .reshape(N, N)
        .astype(np.float32)
        / N
    )
    cnt = np.bincount(recv, minlength=N).astype(np.float32) / N

    # packed per-core input [2, 128, 385]
    packed = np.empty((B, 2, P, W), dtype=np.float32)
    packed[:, :, :, 0:D] = x.reshape(B, 2, P, D)
    packed[:, :, :, D : D + N] = atc.reshape(2, P, N)[None]
    packed[:, :, :, W - 1] = cnt.reshape(2, P)[None]

    if _PROGRAM is None:
        _PROGRAM = _build_program()
    nc = _PROGRAM

    in_maps = [{"inp": np.ascontiguousarray(packed[b])} for b in range(B)]
    res = run_bass_kernel_spmd(nc, in_maps, core_ids=list(range(N_CORES)))

    out = np.empty((B, N, 2 * D), dtype=np.float32)
    for b in range(B):
        r = res.results[b]
        # o1[p, k, :] holds row 128k+p of x*cnt/N
        out[b, :, 0:D] = r["o1"].transpose(1, 0, 2).reshape(N, D)
        # o2t[d, n] = (A @ x / N)[n, d]
        out[b, :, D : 2 * D] = r["o2t"].T
    return out


# revision 7
# speedup vs baseline: 1.3902x; 1.0993x over previous
"""Trainium2 Bass kernel for nn_BaseGraph_67697274519895 (gnn_message_passing).

Reference computation (B=8, N=256, D=128, E=65280):
    edge_feat = concat([x[:, recv, :], x[:, send, :]], -1)        # [B, E, 2D]
    out = zeros([B, N, 2D]).at[:, recv, :].add(edge_feat) / N

With R/S the one-hot [E, N] incidence matrices of recv/send, the scatter-add
is out = R^T @ concat(R @ x, S @ x) / N, which collapses algebraically:
    out[:, :, :D]  = (R^T R) @ x / N = diag(bincount(recv)) @ x / N
    out[:, :, D:]  = (R^T S) @ x / N = A @ x / N,  A[i, j] = #edges (r=i, s=j)
Valid for arbitrary index arrays. A and the counts are built host-side from
the indices (O(E) bincount); the device runs, per batch element, a
[N, N] @ [N, D] matmul plus a per-node row scale.

Sharding: data-parallel over batch — core b handles x[b]; A and counts are
replicated to all 8 cores. No collectives.

Device-side layout (tuned against the TRN2 instruction cost model — DMA
fixed costs dominate at this size, so everything is packed to minimize DMA
instruction count and maximize per-descriptor contiguity):
  - ONE packed input tensor inp[2, 128, 385]: row r of block k holds
    [x[b, 128k+r, :] | (A^T/N)[128k+r, :] | cnt[128k+r]/N].  2 input DMAs.
  - matmul transposed: psum[d, n] = sum_k x_k^T @ (A^T)_k  (free dim 256,
    single PSUM accumulation group), DMA'd straight from PSUM to DRAM.
  - x*cnt half via one DVE tensor-scalar multiply per k block into a single
    [128, 2, 128] tile, one DMA out.  Host un-transposes/interleaves.
"""

import numpy as np

B, N, D = 8, 256, 128
N_CORES = 8
P = 128
W = D + N + 1  # packed input row: x | A^T/N | cnt/N

MATMUL_F32R = True  # flip to use float32r matmuls (4x faster, lower precision)

_PROGRAM = None


def _build_program():
    import concourse.mybir as mybir
    from concourse import bacc
    from concourse.tile import TileContext

    f32 = mybir.dt.float32
    in_dt = mybir.dt.float32r if MATMUL_F32R else f32
    nc = bacc.Bacc(trn_type="TRN2")

    inp = nc.dram_tensor("inp", [2, P, W], in_dt, kind="ExternalInput")
    o1 = nc.dram_tensor("o1", [P, 2, D], f32, kind="ExternalOutput")
    o2t = nc.dram_tensor("o2t", [D, N], f32, kind="ExternalOutput")

    with TileContext(nc) as tc:
        with (
            tc.tile_pool(name="sbuf", bufs=1) as pool,
            tc.tile_pool(name="psum", bufs=1, space="PSUM") as psum_pool,
        ):
            tk = []
            for k in range(2):
                t = pool.tile([P, W], in_dt, tag=f"in{k}", name=f"in{k}")
                nc.sync.dma_start(out=t[:], in_=inp[k])
                tk.append(t)

            # out2^T[d, n] = sum over sender blocks k of x_k^T @ (A^T)_k
            ps = psum_pool.tile([P, N], f32, name="ps")
            for k in range(2):
                lhsT = tk[k][:, 0:D]  # [128 senders, 128 feat] (stationary)
                rhs = tk[k][:, D : D + N]  # [128 senders, 256 receivers]
                nc.tensor.matmul(ps[:], lhsT, rhs, start=(k == 0), stop=(k == 1))

            # out1[p, k, :] = x[128k+p, :] * cnt[128k+p]/N
            ot1 = pool.tile([P, 2, D], f32, name="ot1")
            for k in range(2):
                nc.vector.tensor_scalar_mul(
                    ot1[:, k, :],
                    tk[k][:, 0:D].bitcast(f32),
                    tk[k][:, W - 1 : W].bitcast(f32),
                )

            nc.sync.dma_start(out=o1[:], in_=ot1[:])
            ot2 = pool.tile([P, N], f32, name="ot2")
            nc.vector.tensor_copy(ot2[:], ps[:])
            nc.sync.dma_start(out=o2t[:], in_=ot2[:])

    nc.compile()
    return nc


def kernel(x, receivers, senders):
    global _PROGRAM
    from concourse.bass_utils import run_bass_kernel_spmd

    x = np.ascontiguousarray(np.asarray(x), dtype=np.float32)
    recv = np.asarray(receivers).astype(np.int64).ravel()
    send = np.asarray(senders).astype(np.int64).ravel()
    assert x.shape == (B, N, D), x.shape

    # A^T[s, r] = #edges with (receiver=r, sender=s); scaled by 1/N (exact, N=2^8)
    atc = (
        np.bincount(send * N + recv, minlength=N * N)
        .reshape(N, N)
        .astype(np.float32)
        / N
    )
    cnt = np.bincount(recv, minlength=N).astype(np.float32) / N

    # packed per-core input [2, 128, 385]
    packed = np.empty((B, 2, P, W), dtype=np.float32)
    packed[:, :, :, 0:D] = x.reshape(B, 2, P, D)
    packed[:, :, :, D : D + N] = atc.reshape(2, P, N)[None]
    packed[:, :, :, W - 1] = cnt.reshape(2, P)[None]

    if _PROGRAM is None:
        _PROGRAM = _build_program()
    nc = _PROGRAM

    in_maps = [{"inp": np.ascontiguousarray(packed[b])} for b in range(B)]
    res = run_bass_kernel_spmd(nc, in_maps, core_ids=list(range(N_CORES)))

    out = np.empty((B, N, 2 * D), dtype=np.float32)
    for b in range(B):
        r = res.results[b]
        # o1[p, k, :] holds row 128k+p of x*cnt/N
        out[b, :, 0:D] = r["o1"].transpose(1, 0, 2).reshape(N, D)
        # o2t[d, n] = (A @ x / N)[n, d]
        out[b, :, D : 2 * D] = r["o2t"].T
    return out
